# revision 1
# baseline (speedup 1.0000x reference)
"""MoE grouped-linear (ragged matmul + bias) on 8 TRN2 NeuronCores.

Expert-parallel sharding: core e computes tokens of expert e:
    out_e = X_e[cap, 2048] @ W_e[2048, 8192] + bias
Tokens are pre-sorted by expert (contiguous groups), so the "all-to-all"
is a free host-side slice/concat. No on-device collectives.

Per-core kernel: X_e^T fully resident in SBUF (loaded in per-mi 1MB
slices on the gpsimd DMA queue so the first matmul group starts ~13us
in), W_e streamed in 512-col chunks on the sync queue, PSUM
accumulation over K=16 k-tiles, bias fused into the PSUM eviction on
the vector engine. Matmuls run in float32r (TF32-like multiply, fp32
accumulate: 1 cycle/row for moving dim >= 256 vs 4 cycles/row for
plain fp32; rel err ~1.6e-4 on this problem).

Measured on TRN2 via reps-slope (NTFF profiling unavailable under
axon): ~575-580 us/core, vs a ~573 us streaming floor at the observed
~2.0 GHz effective PE clock (512-row matmul ~258 ns + ~22 ns issue
overhead; weight-load count does not matter — verified by a
same-stationary variant). bf16 measures the same, fp8 would halve it
but fails accuracy.
"""

import numpy as np

E, IN, OUT = 8, 2048, 8192
P = 128
NTILE = 512

_cache = {}


def _build(cap, dtype_name="float32r", reps=1, mode="full", ntile=None):
    import contextlib

    import concourse.mybir as mybir
    import concourse.tile as tile
    from concourse import bacc

    mm_dt = getattr(mybir.dt, dtype_name)
    nt = ntile or NTILE
    KT = IN // P            # 16 k-tiles
    MT = cap // P           # m-tiles per core
    NT = OUT // nt          # n-tiles

    nc = bacc.Bacc(None, target_bir_lowering=False, debug=False)
    with tile.TileContext(nc) as tc:
        with tc.tile_pool(name="dram", bufs=1, space="DRAM") as dram:
            # xt_d[mi, p, k, j] = X[mi*P + j, k*P + p] — per-mi contiguous
            # 1MB slices so the first matmul group can start after ~1MB of DMA
            xt_d = dram.tile((MT, P, KT, P), mm_dt, kind="ExternalInput")
            w_d = dram.tile((P, KT, OUT), mm_dt, kind="ExternalInput")
            bias_d = dram.tile((P, OUT), mybir.dt.float32, kind="ExternalInput")
            out_d = dram.tile((P, MT, OUT), mybir.dt.float32, kind="ExternalOutput")

            with tc.tile_pool(name="resident", bufs=1) as res_pool, \
                 tc.tile_pool(name="wchunk", bufs=2) as w_pool, \
                 tc.tile_pool(name="evict", bufs=6) as o_pool, \
                 tc.tile_pool(name="acc", bufs=6, space="PSUM") as ps_pool:
                loop = tc.For_i(0, reps, 1) if reps > 1 else contextlib.nullcontext()
                with loop:
                    # W stream owns the sync queue; X^T + bias load in
                    # parallel on the gpsimd queue, first-needed first.
                    w_sbs = [None] * NT
                    w_sbs[0] = w_pool.tile([P, KT, nt], mm_dt, tag="w",
                                           name="w_sb0")
                    nc.sync.dma_start(w_sbs[0][:], w_d[:, :, 0:nt])

                    xt_sb = [res_pool.tile([P, KT, P], mm_dt, tag=f"xt{mi}",
                                           name=f"xt_sb{mi}")
                             for mi in range(MT)]
                    bias_sb = res_pool.tile([P, OUT], mybir.dt.float32)
                    nc.gpsimd.dma_start(xt_sb[0][:], xt_d[0])
                    nc.gpsimd.dma_start(bias_sb[:], bias_d[:])
                    for mi in range(1, MT):
                        nc.gpsimd.dma_start(xt_sb[mi][:], xt_d[mi])

                    for ni in range(NT):
                        w_sb = w_sbs[ni]
                        if w_sb is None and mode in ("mm_only", "same_w"):
                            w_sb = w_sbs[0]
                        elif w_sb is None:
                            w_sb = w_pool.tile([P, KT, nt], mm_dt, tag="w",
                                               name=f"w_sb{ni}")
                            nc.sync.dma_start(
                                w_sb[:], w_d[:, :, ni * nt:(ni + 1) * nt])
                        for mi in range(MT):
                            ps = ps_pool.tile([P, nt], mybir.dt.float32)
                            for k in range(KT):
                                nc.tensor.matmul(
                                    ps[:],
                                    lhsT=xt_sb[0][:, 0, :] if mode == "same_w"
                                    else xt_sb[mi][:, k, :],
                                    rhs=w_sb[:, k, :],
                                    start=(k == 0),
                                    stop=(k == KT - 1),
                                )
                            if mode in ("mm_only", "same_w") and not (ni == NT - 1 and mi == MT - 1):
                                continue
                            o_sb = o_pool.tile([P, nt], mybir.dt.float32)
                            nc.vector.tensor_add(
                                out=o_sb[:], in0=ps[:],
                                in1=bias_sb[:, ni * nt:(ni + 1) * nt])
                            nc.sync.dma_start(
                                out_d[:, mi, ni * nt:(ni + 1) * nt], o_sb[:])
    nc.compile()
    names = dict(xt=xt_d.name, w=w_d.name, bias=bias_d.name, out=out_d.name)
    return nc, names


def _get(cap, dtype_name="float32r", reps=1, mode="full", ntile=None):
    key = (cap, dtype_name, reps, mode, ntile)
    if key not in _cache:
        _cache[key] = _build(cap, dtype_name, reps, mode, ntile)
    return _cache[key]


def kernel(inputs, weight, group_sizes, bias):
    from concourse.bass_utils import run_bass_kernel_spmd

    M = inputs.shape[0]
    gs = np.asarray(group_sizes, dtype=np.int64)
    # per-token expert id exactly as the reference's jnp.repeat(...,
    # total_repeat_length=M): truncate or pad with the last expert id
    ids = np.repeat(np.arange(E), gs)
    ids = ids[:M] if len(ids) >= M else np.concatenate(
        [ids, np.full(M - len(ids), E - 1)])
    counts = np.bincount(ids, minlength=E)
    starts = np.concatenate([[0], np.cumsum(counts)])[:E]

    cap = max(P, int(-(-counts.max() // P) * P))
    nc, names = _get(cap)

    x = np.ascontiguousarray(inputs, dtype=np.float32)
    w = np.ascontiguousarray(weight, dtype=np.float32)
    bias_rep = np.ascontiguousarray(
        np.broadcast_to(np.asarray(bias, np.float32), (P, OUT)))

    in_maps = []
    for e in range(E):
        xe = x[starts[e]:starts[e] + counts[e]]
        if xe.shape[0] < cap:
            xe = np.concatenate(
                [xe, np.zeros((cap - xe.shape[0], IN), np.float32)])
        # [cap, IN] -> (MT, P, KT, P): xt[mi, p, k, j] = X[mi*P+j, k*P+p]
        xt = np.ascontiguousarray(
            xe.reshape(cap // P, P, IN // P, P).transpose(0, 3, 2, 1))
        # [IN, OUT] -> (P, KT, OUT): wt[p, a, n] = W[a*P+p, n]
        we = np.ascontiguousarray(
            w[e].reshape(IN // P, P, OUT).transpose(1, 0, 2))
        in_maps.append({names["xt"]: xt, names["w"]: we,
                        names["bias"]: bias_rep})

    res = run_bass_kernel_spmd(nc, in_maps, core_ids=list(range(E)))
    out = np.empty((M, OUT), dtype=np.float32)
    for e in range(E):
        oe = res.results[e][names["out"]]          # (P, cap//P, OUT)
        oe = oe.transpose(1, 0, 2).reshape(cap, OUT)
        out[starts[e]:starts[e] + counts[e]] = oe[:counts[e]]
    return out



# revision 10
# speedup vs baseline: 1.1818x; 1.1818x over previous
"""MoE grouped-linear (ragged matmul + bias) on 8 TRN2 NeuronCores.

Expert-parallel sharding: core e computes tokens of expert e:
    out_e = X_e[cap, 2048] @ W_e[2048, 8192] + bias
Tokens are pre-sorted by expert (contiguous groups), so the "all-to-all"
is a free host-side slice/concat. No on-device collectives.

Per-core kernel: X_e^T fully resident in SBUF (loaded in per-mi 1MB
slices on the gpsimd DMA queue so the first matmul group starts ~13us
in), W_e streamed in 512-col chunks on the sync queue, PSUM
accumulation over K=16 k-tiles, bias fused into the PSUM eviction on
the vector engine. Matmuls run in float32r (TF32-like multiply, fp32
accumulate: 1 cycle/row for moving dim >= 256 vs 4 cycles/row for
plain fp32; rel err ~1.6e-4 on this problem).

Measured on TRN2 via reps-slope (NTFF profiling unavailable under
axon): ~575-580 us/core, vs a ~573 us streaming floor at the observed
~2.0 GHz effective PE clock (512-row matmul ~258 ns + ~22 ns issue
overhead; weight-load count does not matter — verified by a
same-stationary variant). bf16 measures the same, fp8 would halve it
but fails accuracy.
"""

import numpy as np

E, IN, OUT = 8, 2048, 8192
P = 128
NTILE = 512

_cache = {}


def _build(cap, dtype_name="float32r", reps=1, mode="full", ntile=None):
    import contextlib

    import concourse.mybir as mybir
    import concourse.tile as tile
    from concourse import bacc

    mm_dt = getattr(mybir.dt, dtype_name)
    nt = ntile or NTILE
    KT = IN // P            # 16 k-tiles
    MT = cap // P           # m-tiles per core
    NT = OUT // nt          # n-tiles

    nc = bacc.Bacc(None, target_bir_lowering=False, debug=False)
    with tile.TileContext(nc) as tc:
        with tc.tile_pool(name="dram", bufs=1, space="DRAM") as dram:
            # xt_d[mi, p, k, j] = X[mi*P + j, k*P + p] — per-mi contiguous
            # 1MB slices so the first matmul group can start after ~1MB of DMA
            xt_d = dram.tile((MT, P, KT, P), mm_dt, kind="ExternalInput")
            w_d = dram.tile((P, KT, OUT), mm_dt, kind="ExternalInput")
            bias_d = dram.tile((P, OUT), mybir.dt.float32, kind="ExternalInput")
            out_d = dram.tile((P, MT, OUT), mybir.dt.float32, kind="ExternalOutput")

            with tc.tile_pool(name="resident", bufs=1) as res_pool, \
                 tc.tile_pool(name="wchunk", bufs=2) as w_pool, \
                 tc.tile_pool(name="evict", bufs=6) as o_pool, \
                 tc.tile_pool(name="acc", bufs=(3 if nt > 512 else 6), space="PSUM") as ps_pool:
                loop = tc.For_i(0, reps, 1) if reps > 1 else contextlib.nullcontext()
                with loop:
                    # W stream owns the sync queue; X^T + bias load in
                    # parallel on the gpsimd queue, first-needed first.
                    w_sbs = [None] * NT
                    w_sbs[0] = w_pool.tile([P, KT, nt], mm_dt, tag="w",
                                           name="w_sb0")
                    nc.sync.dma_start(w_sbs[0][:], w_d[:, :, 0:nt])

                    xt_sb = [res_pool.tile([P, KT, P], mm_dt, tag=f"xt{mi}",
                                           name=f"xt_sb{mi}")
                             for mi in range(MT)]
                    bias_sb = res_pool.tile([P, OUT], mybir.dt.float32)
                    nc.gpsimd.dma_start(xt_sb[0][:], xt_d[0])
                    nc.gpsimd.dma_start(bias_sb[:], bias_d[:])
                    for mi in range(1, MT):
                        nc.gpsimd.dma_start(xt_sb[mi][:], xt_d[mi])

                    for ni in range(NT):
                        w_sb = w_sbs[ni]
                        if w_sb is None and mode in ("mm_only", "same_w"):
                            w_sb = w_sbs[0]
                        elif w_sb is None:
                            w_sb = w_pool.tile([P, KT, nt], mm_dt, tag="w",
                                               name=f"w_sb{ni}")
                            nc.sync.dma_start(
                                w_sb[:], w_d[:, :, ni * nt:(ni + 1) * nt])
                        for mi in range(MT):
                            ps = ps_pool.tile([P, nt], mybir.dt.float32)
                            for k in range(KT):
                                nc.tensor.matmul(
                                    ps[:],
                                    lhsT=xt_sb[0][:, 0, :] if mode == "same_w"
                                    else xt_sb[mi][:, k, :],
                                    rhs=w_sb[:, k, :],
                                    start=(k == 0),
                                    stop=(k == KT - 1),
                                )
                            if mode in ("mm_only", "same_w") and not (ni == NT - 1 and mi == MT - 1):
                                continue
                            o_sb = o_pool.tile([P, nt], mybir.dt.float32)
                            nc.vector.tensor_add(
                                out=o_sb[:], in0=ps[:],
                                in1=bias_sb[:, ni * nt:(ni + 1) * nt])
                            nc.sync.dma_start(
                                out_d[:, mi, ni * nt:(ni + 1) * nt], o_sb[:])
    nc.compile()
    names = dict(xt=xt_d.name, w=w_d.name, bias=bias_d.name, out=out_d.name)
    return nc, names


def _get(cap, dtype_name="float32r", reps=1, mode="full", ntile=None):
    key = (cap, dtype_name, reps, mode, ntile)
    if key not in _cache:
        _cache[key] = _build(cap, dtype_name, reps, mode, ntile)
    return _cache[key]


def _build2(cap, dtype_name="bfloat16", reps=1, nblk=4, psum_bufs=8,
            mode="full"):
    """LDW-amortized variant: loop (ni_blk, mi, k, ni-in-blk) so each
    stationary x^T[mi,k] serves `nblk` consecutive 512-col matmuls.
    W is streamed once, in [128, KT, nblk*512] blocks, per-k-slice DMAs.
    mode: full | no_evict (only last gen evicts) | same_w (fixed stationary)
    """
    import contextlib

    import concourse.mybir as mybir
    import concourse.tile as tile
    from concourse import bacc

    mm_dt = getattr(mybir.dt, dtype_name)
    nt = NTILE                      # 512
    KT = IN // P                    # 16
    MT = cap // P                   # m-tiles
    NBLK = OUT // (nblk * nt)       # blocks of nblk n-tiles
    bw = nblk * nt                  # block width in cols

    nc = bacc.Bacc(None, target_bir_lowering=False, debug=False)
    with tile.TileContext(nc) as tc:
        with tc.tile_pool(name="dram", bufs=1, space="DRAM") as dram:
            xt_d = dram.tile((MT, P, KT, P), mm_dt, kind="ExternalInput")
            w_d = dram.tile((P, KT, OUT), mm_dt, kind="ExternalInput")
            bias_d = dram.tile((P, OUT), mybir.dt.float32, kind="ExternalInput")
            out_d = dram.tile((P, MT, OUT), mybir.dt.float32, kind="ExternalOutput")

            with tc.tile_pool(name="resident", bufs=1) as res_pool, \
                 tc.tile_pool(name="wblk", bufs=2) as w_pool, \
                 tc.tile_pool(name="evict", bufs=4) as o_pool, \
                 tc.tile_pool(name="acc", bufs=psum_bufs, space="PSUM") as ps_pool:
                loop = tc.For_i(0, reps, 1) if reps > 1 else contextlib.nullcontext()
                with loop:
                    w_sbs = [None] * NBLK
                    w_sbs[0] = w_pool.tile([P, KT, bw], mm_dt, tag="w",
                                           name="w_sb0")
                    # per-k-slice DMAs so the first matmul is gated on
                    # one k-slice, not the whole 8MB block
                    for k in range(KT):
                        nc.sync.dma_start(w_sbs[0][:, k], w_d[:, k, 0:bw])

                    xt_sb = [res_pool.tile([P, KT, P], mm_dt, tag=f"xt{mi}",
                                           name=f"xt_sb{mi}")
                             for mi in range(MT)]
                    bias_sb = res_pool.tile([P, OUT], mybir.dt.float32)
                    nc.gpsimd.dma_start(xt_sb[0][:], xt_d[0])
                    nc.gpsimd.dma_start(bias_sb[:], bias_d[:])
                    for mi in range(1, MT):
                        nc.gpsimd.dma_start(xt_sb[mi][:], xt_d[mi])

                    for blk in range(NBLK):
                        w_sb = w_sbs[blk]
                        if w_sb is None:
                            w_sb = w_pool.tile([P, KT, bw], mm_dt, tag="w",
                                               name=f"w_sb{blk}")
                            for k in range(KT):
                                nc.sync.dma_start(
                                    w_sb[:, k],
                                    w_d[:, k, blk * bw:(blk + 1) * bw])
                        for mi in range(MT):
                            pss = [ps_pool.tile([P, nt], mybir.dt.float32,
                                                tag="ps",
                                                name=f"ps{blk}_{mi}_{j}")
                                   for j in range(nblk)]
                            for k in range(KT):
                                for ni in range(nblk):
                                    nc.tensor.matmul(
                                        pss[ni][:],
                                        lhsT=xt_sb[0][:, 0, :] if mode == "same_w"
                                        else xt_sb[mi][:, k, :],
                                        rhs=w_sb[:, k, ni * nt:(ni + 1) * nt],
                                        start=(k == 0),
                                        stop=(k == KT - 1),
                                    )
                            if mode in ("no_evict", "same_w") and not (
                                    blk == NBLK - 1 and mi == MT - 1):
                                continue
                            for ni in range(nblk):
                                o_sb = o_pool.tile([P, nt], mybir.dt.float32)
                                col0 = blk * bw + ni * nt
                                nc.vector.tensor_add(
                                    out=o_sb[:], in0=pss[ni][:],
                                    in1=bias_sb[:, col0:col0 + nt])
                                nc.sync.dma_start(
                                    out_d[:, mi, col0:col0 + nt], o_sb[:])
    nc.compile()
    names = dict(xt=xt_d.name, w=w_d.name, bias=bias_d.name, out=out_d.name)
    return nc, names


def _get2(cap, dtype_name="bfloat16", reps=1, nblk=4, psum_bufs=8,
          mode="full"):
    key = ("v2", cap, dtype_name, reps, nblk, psum_bufs, mode)
    if key not in _cache:
        _cache[key] = _build2(cap, dtype_name, reps, nblk, psum_bufs, mode)
    return _cache[key]


def kernel(inputs, weight, group_sizes, bias):
    import ml_dtypes

    from concourse.bass_utils import run_bass_kernel_spmd

    M = inputs.shape[0]
    gs = np.asarray(group_sizes, dtype=np.int64)
    # per-token expert id exactly as the reference's jnp.repeat(...,
    # total_repeat_length=M): truncate or pad with the last expert id
    ids = np.repeat(np.arange(E), gs)
    ids = ids[:M] if len(ids) >= M else np.concatenate(
        [ids, np.full(M - len(ids), E - 1)])
    counts = np.bincount(ids, minlength=E)
    starts = np.concatenate([[0], np.cumsum(counts)])[:E]

    cap = max(P, int(-(-counts.max() // P) * P))
    nc, names = _get2(cap)

    x = np.asarray(inputs, dtype=np.float32).astype(ml_dtypes.bfloat16)
    w = np.asarray(weight, dtype=np.float32).astype(ml_dtypes.bfloat16)
    bias_rep = np.ascontiguousarray(
        np.broadcast_to(np.asarray(bias, np.float32), (P, OUT)))

    in_maps = []
    for e in range(E):
        xe = x[starts[e]:starts[e] + counts[e]]
        if xe.shape[0] < cap:
            xe = np.concatenate(
                [xe, np.zeros((cap - xe.shape[0], IN), ml_dtypes.bfloat16)])
        # [cap, IN] -> (MT, P, KT, P): xt[mi, p, k, j] = X[mi*P+j, k*P+p]
        xt = np.ascontiguousarray(
            xe.reshape(cap // P, P, IN // P, P).transpose(0, 3, 2, 1))
        # [IN, OUT] -> (P, KT, OUT): wt[p, a, n] = W[a*P+p, n]
        we = np.ascontiguousarray(
            w[e].reshape(IN // P, P, OUT).transpose(1, 0, 2))
        in_maps.append({names["xt"]: xt, names["w"]: we,
                        names["bias"]: bias_rep})

    res = run_bass_kernel_spmd(nc, in_maps, core_ids=list(range(E)))
    out = np.empty((M, OUT), dtype=np.float32)
    for e in range(E):
        oe = res.results[e][names["out"]]          # (P, cap//P, OUT)
        oe = oe.transpose(1, 0, 2).reshape(cap, OUT)
        out[starts[e]:starts[e] + counts[e]] = oe[:counts[e]]
    return out



# revision 11
# speedup vs baseline: 1.3226x; 1.1192x over previous
"""MoE grouped-linear (ragged matmul + bias) on 8 TRN2 NeuronCores.

Expert-parallel sharding: core e computes tokens of expert e:
    out_e = X_e[cap, 2048] @ W_e[2048, 8192] + bias
Tokens are pre-sorted by expert (contiguous groups), so the "all-to-all"
is a free host-side slice/concat. No on-device collectives.

Per-core kernel (_build2, the production path): bf16 matmuls (rel err
2.4e-3 on this problem, gate is 2e-2), loop order (n-block of 4, mi, k,
ni-in-block) so each stationary x^T[mi,k] serves 4 consecutive 512-col
matmuls — this amortizes/hides LDWEIGHTS, which at ~107 ns per 128-col
load is NOT hidden when the stationary changes every matmul (the old
fp32r kernel's ~258 ns/MM vs the 213.3 ns = 512row/2.4GHz warm floor).
All 8 PSUM banks cycle through the 4 concurrent accumulation chains;
bias is fused into the PSUM eviction on the vector engine; W streams
once in [128,16,2048] blocks with per-k-slice DMAs; X^T + bias load on
the gpsimd queue.

Measured via reps-slope R=1 vs R=33 (no NTFF under axon): ~435-440
us/core in quiet windows = the 2048x213.3ns tensor-engine floor at the
full 2.4 GHz clock (same-stationary microbench hits 219 ns/MM, proving
the clock and that LDW was the old kernel's gap). Under sustained load
or tenant/thermal contention the package duty-cycles the PE clock
(K=4/8 HAM states -> ~1.8-2.0 GHz effective) and ALL kernel variants
(incl. no-evict / same-stationary) converge to ~540-585 us — that
regime is power-bound, not structure-bound. fp8 e4m3 DoubleRow (the
only >1x mode) fails accuracy: 4.0e-2 single-pass, 2.7e-2 even with a
2-pass hi/lo split of either operand (e4m3's ~2.5% per-element error
x sqrt(K=2048) accumulation); 3-pass passes but is slower than bf16.
nt=1024 moving is rejected by the walrus birverifier (512 max).
"""

import numpy as np

E, IN, OUT = 8, 2048, 8192
P = 128
NTILE = 512

_cache = {}


def _build(cap, dtype_name="float32r", reps=1, mode="full", ntile=None):
    import contextlib

    import concourse.mybir as mybir
    import concourse.tile as tile
    from concourse import bacc

    mm_dt = getattr(mybir.dt, dtype_name)
    nt = ntile or NTILE
    KT = IN // P            # 16 k-tiles
    MT = cap // P           # m-tiles per core
    NT = OUT // nt          # n-tiles

    nc = bacc.Bacc(None, target_bir_lowering=False, debug=False)
    with tile.TileContext(nc) as tc:
        with tc.tile_pool(name="dram", bufs=1, space="DRAM") as dram:
            # xt_d[mi, p, k, j] = X[mi*P + j, k*P + p] — per-mi contiguous
            # 1MB slices so the first matmul group can start after ~1MB of DMA
            xt_d = dram.tile((MT, P, KT, P), mm_dt, kind="ExternalInput")
            w_d = dram.tile((P, KT, OUT), mm_dt, kind="ExternalInput")
            bias_d = dram.tile((P, OUT), mybir.dt.float32, kind="ExternalInput")
            out_d = dram.tile((P, MT, OUT), mybir.dt.float32, kind="ExternalOutput")

            with tc.tile_pool(name="resident", bufs=1) as res_pool, \
                 tc.tile_pool(name="wchunk", bufs=2) as w_pool, \
                 tc.tile_pool(name="evict", bufs=6) as o_pool, \
                 tc.tile_pool(name="acc", bufs=(3 if nt > 512 else 6), space="PSUM") as ps_pool:
                loop = tc.For_i(0, reps, 1) if reps > 1 else contextlib.nullcontext()
                with loop:
                    # W stream owns the sync queue; X^T + bias load in
                    # parallel on the gpsimd queue, first-needed first.
                    w_sbs = [None] * NT
                    w_sbs[0] = w_pool.tile([P, KT, nt], mm_dt, tag="w",
                                           name="w_sb0")
                    nc.sync.dma_start(w_sbs[0][:], w_d[:, :, 0:nt])

                    xt_sb = [res_pool.tile([P, KT, P], mm_dt, tag=f"xt{mi}",
                                           name=f"xt_sb{mi}")
                             for mi in range(MT)]
                    bias_sb = res_pool.tile([P, OUT], mybir.dt.float32)
                    nc.gpsimd.dma_start(xt_sb[0][:], xt_d[0])
                    nc.gpsimd.dma_start(bias_sb[:], bias_d[:])
                    for mi in range(1, MT):
                        nc.gpsimd.dma_start(xt_sb[mi][:], xt_d[mi])

                    for ni in range(NT):
                        w_sb = w_sbs[ni]
                        if w_sb is None and mode in ("mm_only", "same_w"):
                            w_sb = w_sbs[0]
                        elif w_sb is None:
                            w_sb = w_pool.tile([P, KT, nt], mm_dt, tag="w",
                                               name=f"w_sb{ni}")
                            nc.sync.dma_start(
                                w_sb[:], w_d[:, :, ni * nt:(ni + 1) * nt])
                        for mi in range(MT):
                            ps = ps_pool.tile([P, nt], mybir.dt.float32)
                            for k in range(KT):
                                nc.tensor.matmul(
                                    ps[:],
                                    lhsT=xt_sb[0][:, 0, :] if mode == "same_w"
                                    else xt_sb[mi][:, k, :],
                                    rhs=w_sb[:, k, :],
                                    start=(k == 0),
                                    stop=(k == KT - 1),
                                )
                            if mode in ("mm_only", "same_w") and not (ni == NT - 1 and mi == MT - 1):
                                continue
                            o_sb = o_pool.tile([P, nt], mybir.dt.float32)
                            nc.vector.tensor_add(
                                out=o_sb[:], in0=ps[:],
                                in1=bias_sb[:, ni * nt:(ni + 1) * nt])
                            nc.sync.dma_start(
                                out_d[:, mi, ni * nt:(ni + 1) * nt], o_sb[:])
    nc.compile()
    names = dict(xt=xt_d.name, w=w_d.name, bias=bias_d.name, out=out_d.name)
    return nc, names


def _get(cap, dtype_name="float32r", reps=1, mode="full", ntile=None):
    key = (cap, dtype_name, reps, mode, ntile)
    if key not in _cache:
        _cache[key] = _build(cap, dtype_name, reps, mode, ntile)
    return _cache[key]


def _build2(cap, dtype_name="bfloat16", reps=1, nblk=4, psum_bufs=8,
            mode="full"):
    """LDW-amortized variant: loop (ni_blk, mi, k, ni-in-blk) so each
    stationary x^T[mi,k] serves `nblk` consecutive 512-col matmuls.
    W is streamed once, in [128, KT, nblk*512] blocks, per-k-slice DMAs.
    mode: full | no_evict (only last gen evicts) | same_w (fixed stationary)
    """
    import contextlib

    import concourse.mybir as mybir
    import concourse.tile as tile
    from concourse import bacc

    mm_dt = getattr(mybir.dt, dtype_name)
    nt = NTILE                      # 512
    KT = IN // P                    # 16
    MT = cap // P                   # m-tiles
    NBLK = OUT // (nblk * nt)       # blocks of nblk n-tiles
    bw = nblk * nt                  # block width in cols

    nc = bacc.Bacc(None, target_bir_lowering=False, debug=False)
    with tile.TileContext(nc) as tc:
        with tc.tile_pool(name="dram", bufs=1, space="DRAM") as dram:
            xt_d = dram.tile((MT, P, KT, P), mm_dt, kind="ExternalInput")
            w_d = dram.tile((P, KT, OUT), mm_dt, kind="ExternalInput")
            bias_d = dram.tile((P, OUT), mybir.dt.float32, kind="ExternalInput")
            out_d = dram.tile((P, MT, OUT), mybir.dt.float32, kind="ExternalOutput")

            with tc.tile_pool(name="resident", bufs=1) as res_pool, \
                 tc.tile_pool(name="wblk", bufs=2) as w_pool, \
                 tc.tile_pool(name="evict", bufs=4) as o_pool, \
                 tc.tile_pool(name="acc", bufs=psum_bufs, space="PSUM") as ps_pool:
                loop = tc.For_i(0, reps, 1) if reps > 1 else contextlib.nullcontext()
                with loop:
                    w_sbs = [None] * NBLK
                    w_sbs[0] = w_pool.tile([P, KT, bw], mm_dt, tag="w",
                                           name="w_sb0")
                    # per-k-slice DMAs so the first matmul is gated on
                    # one k-slice, not the whole 8MB block
                    for k in range(KT):
                        nc.sync.dma_start(w_sbs[0][:, k], w_d[:, k, 0:bw])

                    xt_sb = [res_pool.tile([P, KT, P], mm_dt, tag=f"xt{mi}",
                                           name=f"xt_sb{mi}")
                             for mi in range(MT)]
                    bias_sb = res_pool.tile([P, OUT], mybir.dt.float32)
                    nc.gpsimd.dma_start(xt_sb[0][:], xt_d[0])
                    nc.gpsimd.dma_start(bias_sb[:], bias_d[:])
                    for mi in range(1, MT):
                        nc.gpsimd.dma_start(xt_sb[mi][:], xt_d[mi])

                    for blk in range(NBLK):
                        w_sb = w_sbs[blk]
                        if w_sb is None:
                            w_sb = w_pool.tile([P, KT, bw], mm_dt, tag="w",
                                               name=f"w_sb{blk}")
                            for k in range(KT):
                                nc.sync.dma_start(
                                    w_sb[:, k],
                                    w_d[:, k, blk * bw:(blk + 1) * bw])
                        for mi in range(MT):
                            pss = [ps_pool.tile([P, nt], mybir.dt.float32,
                                                tag="ps",
                                                name=f"ps{blk}_{mi}_{j}")
                                   for j in range(nblk)]
                            for k in range(KT):
                                for ni in range(nblk):
                                    nc.tensor.matmul(
                                        pss[ni][:],
                                        lhsT=xt_sb[0][:, 0, :] if mode == "same_w"
                                        else xt_sb[mi][:, k, :],
                                        rhs=w_sb[:, k, ni * nt:(ni + 1) * nt],
                                        start=(k == 0),
                                        stop=(k == KT - 1),
                                    )
                            if mode in ("no_evict", "same_w") and not (
                                    blk == NBLK - 1 and mi == MT - 1):
                                continue
                            for ni in range(nblk):
                                o_sb = o_pool.tile([P, nt], mybir.dt.float32)
                                col0 = blk * bw + ni * nt
                                nc.vector.tensor_add(
                                    out=o_sb[:], in0=pss[ni][:],
                                    in1=bias_sb[:, col0:col0 + nt])
                                nc.sync.dma_start(
                                    out_d[:, mi, col0:col0 + nt], o_sb[:])
    nc.compile()
    names = dict(xt=xt_d.name, w=w_d.name, bias=bias_d.name, out=out_d.name)
    return nc, names


def _get2(cap, dtype_name="bfloat16", reps=1, nblk=4, psum_bufs=8,
          mode="full"):
    key = ("v2", cap, dtype_name, reps, nblk, psum_bufs, mode)
    if key not in _cache:
        _cache[key] = _build2(cap, dtype_name, reps, nblk, psum_bufs, mode)
    return _cache[key]


def kernel(inputs, weight, group_sizes, bias):
    import ml_dtypes

    from concourse.bass_utils import run_bass_kernel_spmd

    M = inputs.shape[0]
    gs = np.asarray(group_sizes, dtype=np.int64)
    # per-token expert id exactly as the reference's jnp.repeat(...,
    # total_repeat_length=M): truncate or pad with the last expert id
    ids = np.repeat(np.arange(E), gs)
    ids = ids[:M] if len(ids) >= M else np.concatenate(
        [ids, np.full(M - len(ids), E - 1)])
    counts = np.bincount(ids, minlength=E)
    starts = np.concatenate([[0], np.cumsum(counts)])[:E]

    cap = max(P, int(-(-counts.max() // P) * P))
    nc, names = _get2(cap)

    x = np.asarray(inputs, dtype=np.float32).astype(ml_dtypes.bfloat16)
    w = np.asarray(weight, dtype=np.float32).astype(ml_dtypes.bfloat16)
    bias_rep = np.ascontiguousarray(
        np.broadcast_to(np.asarray(bias, np.float32), (P, OUT)))

    in_maps = []
    for e in range(E):
        xe = x[starts[e]:starts[e] + counts[e]]
        if xe.shape[0] < cap:
            xe = np.concatenate(
                [xe, np.zeros((cap - xe.shape[0], IN), ml_dtypes.bfloat16)])
        # [cap, IN] -> (MT, P, KT, P): xt[mi, p, k, j] = X[mi*P+j, k*P+p]
        xt = np.ascontiguousarray(
            xe.reshape(cap // P, P, IN // P, P).transpose(0, 3, 2, 1))
        # [IN, OUT] -> (P, KT, OUT): wt[p, a, n] = W[a*P+p, n]
        we = np.ascontiguousarray(
            w[e].reshape(IN // P, P, OUT).transpose(1, 0, 2))
        in_maps.append({names["xt"]: xt, names["w"]: we,
                        names["bias"]: bias_rep})

    res = run_bass_kernel_spmd(nc, in_maps, core_ids=list(range(E)))
    out = np.empty((M, OUT), dtype=np.float32)
    for e in range(E):
        oe = res.results[e][names["out"]]          # (P, cap//P, OUT)
        oe = oe.transpose(1, 0, 2).reshape(cap, OUT)
        out[starts[e]:starts[e] + counts[e]] = oe[:counts[e]]
    return out

